# revision 27
# baseline (speedup 1.0000x reference)
"""Location-sensitive attention Trainium2 kernel (v4 — group pipeline).

Strategy (data-parallel over batch, 8 cores, B=128 -> 16 per core):
  - encoder shipped in BOTH layouts as bf16 (transposed [E, bt] for the
    projection, natural [bt, E] for the context) — same total HBM bytes
    as one fp32 copy.  Partition-major, 8KB contiguous runs.
  - 16 stream items (8 encT + 8 enc_nat tiles of 2 batches each),
    chained two-deep with tiny WAW "gate" DMAs so at most ~3 transfers
    are in flight: arrival order == issue order at full bandwidth
    (otherwise the SDMA engines round-robin ALL queued transfers and
    everything lands at the end simultaneously).
  - 4-group pipeline (4 batches per group): proj -> group softmax ->
    group context.  Context matmuls of group G fill the PE gaps while
    group G+1's encT streams, keeping the tensor engine dense (the PE
    p-state drops to 1.2 GHz after any idle gap and needs 3us of
    continuous work to reach 2.4 GHz).
  - weights packed host-side into contiguous blocks; decoder projection
    computed on host (tiny, B*D work) and shipped as an 8KB bias.
  - conv1d folded into W_loc on the host (im2col prevrep).
  - energies via per-batch column-masked W_e into [4, 512] group PSUM.
  - context: block-diagonal scattered exp^T tile L so all 64 chunk
    matmuls accumulate into ONE [16, E] PSUM tile; softmax
    normalization folded into the final copy's per-row scale.
  - b_e dropped: softmax is shift-invariant.
"""

import sys

for p in ("/opt/trn_rl_repo",):
    if p not in sys.path:
        sys.path.insert(0, p)

import numpy as np
import ml_dtypes

import concourse.bass as bass
import concourse.tile as tile
from concourse import mybir
from concourse import bacc
from concourse import bass_utils
from concourse.masks import make_identity

BF = ml_dtypes.bfloat16

NCORES = 8
B, T, E, D, A, F, KW = 128, 512, 512, 1024, 128, 32, 31
BS = B // NCORES          # 16 batches per core
NG = 8                    # encT / enc_nat tiles of 2 batches
P = 128


def build_device_program(nc):
    dt = mybir.dt
    f32, bf16 = dt.float32, dt.bfloat16
    Act = mybir.ActivationFunctionType

    encT = nc.dram_tensor("encT", (P, NG, 2, 4, T), bf16, kind="ExternalInput").ap()
    enc_nat = nc.dram_tensor("enc_nat", (P, NG, 8, E), bf16, kind="ExternalInput").ap()
    # [w_encT 4x128 | w_ediag 16x4]
    wpack = nc.dram_tensor("wpack", (P, 4 * A + BS * 4), bf16, kind="ExternalInput").ap()
    # [w_combT 128 | prevrep 8192]
    prevcomb = nc.dram_tensor("prevcomb", (32, A + BS * T), bf16, kind="ExternalInput").ap()
    decp = nc.dram_tensor("decp", (A, BS), f32, kind="ExternalInput").ap()
    ctx_out = nc.dram_tensor("context_out", (BS, E), f32, kind="ExternalOutput").ap()
    attn_out = nc.dram_tensor("attn_out", (BS, T), f32, kind="ExternalOutput").ap()

    with tile.TileContext(nc) as tc:
        with (
            tc.tile_pool(name="const", bufs=1) as const,
            tc.tile_pool(name="big", bufs=1) as big,
            tc.tile_pool(name="work", bufs=1) as work,
            tc.tile_pool(name="ps_pe", bufs=3, space="PSUM") as ps_pe,
            tc.tile_pool(name="ps_en", bufs=2, space="PSUM") as ps_en,
            tc.tile_pool(name="ps_at", bufs=1, space="PSUM") as ps_at,
            tc.tile_pool(name="ps_ctx", bufs=1, space="PSUM") as ps_ctx,
        ):
            # ---- weights / constants: split loads, sync-ring FIFO order
            # matches first-use order so the ~10us DMA slow-start window
            # moves only bytes the projection needs immediately ----
            wpack_sb = const.tile([P, 4 * A + BS * 4], bf16)
            prevcomb_sb = const.tile([32, A + BS * T], bf16)
            decp_sb = const.tile([A, BS], f32)
            ident4 = const.tile([4, 4], bf16)
            make_identity(nc, ident4)

            # L: block-diagonal scattered exp^T, zeroed early; group
            # strips fill after each group softmax.
            L = work.tile([P, 4 * BS, 8], bf16)
            nc.vector.memset(L, 0.0)

            # ---- streams ----
            # First four batches get per-batch half tiles: during the
            # ~10us DMA slow-start only ~1.2 MB arrives, so small lead
            # tiles let the projection start much earlier.
            encTb_sb = [big.tile([P, 4, T], bf16, name=f"encTb{b}", tag=f"encTb{b}")
                        for b in range(4)]
            encT_sb = [None, None] + [
                big.tile([P, 2, 4, T], bf16, name=f"encT{g}", tag=f"encT{g}")
                for g in range(2, NG)]
            nat_sb = [big.tile([P, 8, E], bf16, name=f"nat{g}", tag=f"nat{g}")
                      for g in range(6)]
            natb_sb = [big.tile([P, 4, E], bf16, name=f"natb{b}", tag=f"natb{b}")
                       for b in range(4)]   # batches 12..15
            # all encT first (projection is the program's spine), then the
            # nat stream which the context matmuls chase to the end.
            stream = ([("e", g) for g in range(NG)] + [("n", g) for g in range(NG)])

            def corner(kind, g):
                t = encT_sb[g] if kind == "e" else nat_sb[g]
                return t[0:1, 0, 0, 0:64] if kind == "e" else t[0:1, 0, 0:64]

            # qSPDynamicHW is a single FIFO ring: sync-issued transfers
            # drain in trigger order at full bandwidth, so issue order ==
            # arrival order with no gating needed.
            nc.sync.dma_start(wpack_sb, wpack)
            nc.scalar.dma_start(prevcomb_sb, prevcomb)
            nc.scalar.dma_start(decp_sb, decp)
            for kind, g in stream:
                if kind == "e":
                    if g < 2:
                        for j in range(2):
                            b = 2 * g + j
                            nc.sync.dma_start(encTb_sb[b], encT[:, g, j])
                    else:
                        nc.sync.dma_start(encT_sb[g], encT[:, g])
                elif g < 6:
                    nc.sync.dma_start(nat_sb[g], enc_nat[:, g])
                else:
                    for j in range(2):
                        nc.sync.dma_start(
                            natb_sb[2 * (g - 6) + j],
                            enc_nat[:, g, 4 * j:4 * j + 4],
                        )

            # ---- PE warm-up: the first encoder tile lands ~14us into the
            # run (DMA slow-start) while the weights land ~10us; dummy
            # matmuls on the weight tile keep the PE continuously busy
            # through the wait so the p-state is fully ramped (2.4 GHz)
            # when real work starts.
            for w in range(8):
                dmy = ps_at.tile([4, T], f32, name=f"dmy{w}", tag="attnT")
                nc.tensor.matmul(
                    dmy,
                    lhsT=wpack_sb[:, 0:4],
                    rhs=wpack_sb[:, 0:T],
                    start=True,
                    stop=True,
                )

            # ---- 4-group pipeline ----
            psum_ctx = [ps_ctx.tile([8, E], f32, name="psc0", tag="ctx0"),
                        ps_ctx.tile([8, E], f32, name="psc1", tag="ctx1")]

            # Each group's post-projection block (energies + softmax +
            # exp^T scatter into L) is deferred until after the NEXT
            # group's first projection batch: the last tanh of a group
            # then completes under real PE work instead of stalling it.
            pending = []

            def make_post(G, tanhs, psum_energ):
                def post():
                    for i in range(4):
                        b = 4 * G + i
                        nc.tensor.matmul(
                            psum_energ,
                            lhsT=wpack_sb[:, 4 * A + b * 4:4 * A + (b + 1) * 4],
                            rhs=tanhs[i],
                            start=(i == 0),
                            stop=(i == 3),
                        )
                    # group softmax (rows 4G..4G+3).  No max-subtraction:
                    # energies = w_e . tanh(...) are bounded by ~11, so
                    # exp cannot overflow.
                    exp_g = work.tile([4, T], bf16, name=f"exp{G}", tag="exp",
                                      bufs=2)
                    esum = work.tile([4, 1], f32, name=f"esum{G}", tag="esum",
                                     bufs=2)
                    nc.scalar.activation(
                        exp_g, psum_energ, Act.Exp, scale=1.0, accum_out=esum,
                    )
                    rs = work.tile([4, 1], f32, name=f"rs{G}", tag="rs", bufs=2)
                    nc.vector.reciprocal(rs, esum)
                    attn_g = work.tile([4, T], f32, name=f"attn{G}",
                                       tag="attn_g", bufs=2)
                    nc.vector.tensor_scalar_mul(attn_g, exp_g, rs)
                    nc.sync.dma_start(attn_out[4 * G:4 * G + 4, :], attn_g)
                    # normalized bf16 attention for the context path
                    abf_g = work.tile([4, T], bf16, name=f"abf{G}", tag="abf",
                                      bufs=2)
                    nc.vector.tensor_scalar_mul(abf_g, exp_g, rs)
                    psum_at = ps_at.tile([P, 4, 4], bf16, name=f"pat{G}",
                                         tag="attnT", bufs=1)
                    for q in range(4):
                        nc.tensor.transpose(
                            psum_at[:, q, :], abf_g[:, q * P:(q + 1) * P], ident4
                        )
                    for i in range(4):
                        b = 4 * G + i
                        nc.vector.tensor_copy(
                            L[:, 4 * b:4 * b + 4, (b % 8):(b % 8) + 1],
                            psum_at[:, :, i:i + 1],
                        )
                return post

            for G in range(4):
                psum_energ = ps_en.tile([4, T], f32, name=f"pen{G}",
                                        tag="energ", bufs=2)
                tanhs = []
                for i in range(4):
                    b = 4 * G + i
                    g, j = b // 2, b % 2
                    pe_t = ps_pe.tile([A, T], f32, tag="pe", bufs=3)
                    for et in range(4):
                        rhs = (encTb_sb[b][:, et, :] if b < 4
                               else encT_sb[g][:, j, et, :])
                        nc.tensor.matmul(
                            pe_t,
                            lhsT=wpack_sb[:, et * A:(et + 1) * A],
                            rhs=rhs,
                            start=(et == 0),
                            stop=False,
                        )
                    nc.tensor.matmul(
                        pe_t,
                        lhsT=prevcomb_sb[:, 0:A],
                        rhs=prevcomb_sb[:, A + b * T:A + (b + 1) * T],
                        start=False,
                        stop=True,
                    )
                    tanh_t = work.tile([A, T], bf16, tag="tanh", bufs=8)
                    nc.scalar.activation(
                        tanh_t, pe_t, Act.Tanh, bias=decp_sb[:, b:b + 1], scale=1.0
                    )
                    tanhs.append(tanh_t)
                    if i == 3 and pending:
                        pending.pop(0)()
                pending.append(make_post(G, tanhs, psum_energ))
            while pending:
                pending.pop(0)()

            # context matmuls issued AFTER the whole proj/softmax pipeline:
            # lower priority, so they act as PE gap filler.  Two PSUM
            # halves so the first half's copy + output DMA hide behind
            # the second half's matmuls.
            ctx_ranges = [(0, 32, slice(0, 8)), (32, 64, slice(0, 8))]
            for h, (c0, c1, lsl) in enumerate(ctx_ranges):
                for c in range(c0, c1):
                    j, k = c // 8, c % 8
                    if j < 6:
                        rhs = nat_sb[j][:, k, :]
                    else:
                        rhs = natb_sb[2 * (j - 6) + k // 4][:, k % 4, :]
                    nc.tensor.matmul(
                        psum_ctx[h],
                        lhsT=L[:, c, lsl],
                        rhs=rhs,
                        start=(c == c0),
                        stop=(c == c1 - 1),
                    )
                ctxg = work.tile([8, E], f32, name=f"ctxg{h}", tag=f"ctxg{h}")
                nc.vector.tensor_copy(ctxg, psum_ctx[h])
                nc.sync.dma_start(ctx_out[8 * h:8 * h + 8, :], ctxg)

    return nc


def host_prepare(encoder_outputs, decoder_state, prev_attention_weights,
                 W_enc, W_dec, conv_w, W_loc, W_e, b_e):
    """Build per-core input maps (host-side marshaling, all numpy)."""
    f32 = np.float32
    enc = np.asarray(encoder_outputs, dtype=f32)
    dec = np.asarray(decoder_state, dtype=f32)
    prev = np.asarray(prev_attention_weights, dtype=f32)
    W_enc = np.asarray(W_enc, dtype=f32)
    W_dec = np.asarray(W_dec, dtype=f32)
    conv_w = np.asarray(conv_w, dtype=f32)
    W_loc = np.asarray(W_loc, dtype=f32)
    W_e = np.asarray(W_e, dtype=f32)

    # wpack: [p, 4*A + 64] = [w_encT | w_ediag (16 groups of 4)]
    wpack = np.zeros((P, 4 * A + BS * 4), dtype=BF)
    wpack[:, :4 * A] = (
        W_enc.T.reshape(4, P, A).transpose(1, 0, 2).reshape(P, 4 * A).astype(BF)
    )
    we = W_e[0].astype(BF)                                     # [A]
    for b in range(BS):
        wpack[:, 4 * A + b * 4 + (b % 4)] = we
    Wcomb = W_loc @ conv_w[:, 0, :]                            # [A, KW]
    pp = np.pad(prev, ((0, 0), (15, 15)))                      # [B, T+30]
    decp_full = (W_dec @ dec.T).astype(f32)                    # [A, B]

    in_maps = []
    for c in range(NCORES):
        sl = slice(c * BS, (c + 1) * BS)
        enc_c = enc[sl].astype(BF)                             # [BS, T, E]
        # encT: [p, g, j, et, t] = enc[2g+j, t, et*128+p]
        encT = np.ascontiguousarray(
            enc_c.transpose(2, 0, 1)                           # [E, BS, T]
            .reshape(4, P, NG, 2, T)
            .transpose(1, 2, 3, 0, 4)                          # [p, g, j, et, t]
        )
        # enc_nat: [p, g, k, e] = enc[b, q*128+p, e],  8g+k = 4b+q
        enc_nat = np.ascontiguousarray(
            enc_c.reshape(BS * 4, P, E).transpose(1, 0, 2)     # [p, 64, E]
            .reshape(P, NG, 8, E)
        )
        prevcomb = np.zeros((32, A + BS * T), dtype=BF)
        prevcomb[:KW, :A] = Wcomb.T.astype(BF)
        pc = pp[sl]
        for k in range(KW):
            prevcomb[k, A:] = pc[:, k:k + T].astype(BF).reshape(-1)
        in_maps.append({
            "encT": encT,
            "enc_nat": enc_nat,
            "wpack": wpack,
            "prevcomb": np.ascontiguousarray(prevcomb),
            "decp": np.ascontiguousarray(decp_full[:, sl]),
        })
    return in_maps


_NC_CACHE = {}


def get_nc():
    if "nc" not in _NC_CACHE:
        nc = bacc.Bacc("TRN2", debug=False, num_devices=NCORES)
        build_device_program(nc)
        nc.finalize()
        _NC_CACHE["nc"] = nc
    return _NC_CACHE["nc"]


def kernel(encoder_outputs, decoder_state, prev_attention_weights,
           W_enc, W_dec, conv_w, W_loc, W_e, b_e, _trace=False, _result_box=None):
    in_maps = host_prepare(
        encoder_outputs, decoder_state, prev_attention_weights,
        W_enc, W_dec, conv_w, W_loc, W_e, b_e,
    )
    nc = get_nc()
    res = bass_utils.run_bass_kernel_spmd(
        nc, in_maps, core_ids=list(range(NCORES)), trace=_trace,
    )
    if _result_box is not None:
        _result_box.append(res)
    ctx = np.concatenate([r["context_out"] for r in res.results], axis=0)
    attn = np.concatenate([r["attn_out"] for r in res.results], axis=0)
    return ctx.astype(np.float32), attn.astype(np.float32)
